# revision 13
# baseline (speedup 1.0000x reference)
"""3-layer GCN encoder on 8 TRN2 NeuronCores (Bass/Tile).

Strategy: partition nodes (dst) across 8 cores. Per layer: local matmul
h = prev @ W, pre-scale h' = h * dinv (factors the symmetric norm so the
edge aggregation is an unweighted sum), AllGather h' (fp32 [101376, 64]),
then aggregate in-edges with bulk dma_gather passes: pass (g, k) gathers
the k-th neighbor-in-source-group-g of every node slot (nodes degree-
sorted so each pass covers a slot prefix; holes point at a zero row).
DVE accumulates gathered rows into the fp32 agg tile. Finally
out[d] = dinv[d] * (h'[d] + sum) -> bias/relu -> next layer.
"""
import os
import numpy as np

import concourse.bacc as bacc
import concourse.bass as bass
import concourse.mybir as mybir
import concourse.tile as tile
from concourse.masks import make_identity

F32 = mybir.dt.float32
I16 = mybir.dt.int16

NCORES = 8
P = 128


class Meta:
    pass


def _preprocess(x, edge_index, n_groups=4):
    """Host-side graph preprocessing -> per-core arrays + pass structure."""
    m = Meta()
    N, IN_DIM = x.shape
    E = edge_index.shape[1]
    NPC = N // NCORES                       # real nodes per core
    SLOTS = ((NPC + P - 1) // P) * P        # padded slots per core (mult of 128)
    MBLK = SLOTS // P                       # 98 column blocks
    SLICE = SLOTS + P                       # hp rows per core (+128 zero rows)
    GROUP_CORES = NCORES // n_groups
    GRP_ROWS = GROUP_CORES * SLICE          # hp rows per source group
    assert GRP_ROWS - 1 < 32768, "group rows must fit int16"

    src = np.asarray(edge_index[0], dtype=np.int64)
    dst = np.asarray(edge_index[1], dtype=np.int64)
    deg = np.bincount(dst, minlength=N)
    dinv = 1.0 / np.sqrt(deg + 1.0)

    # per-core slot assignment sorted by MAX per-source-group in-degree:
    # every group-g pass prefix is then bounded by the maxdeg-k prefix,
    # which minimizes zero-fill holes across all groups at once.
    core_of_node_pre = np.arange(N) // NPC
    # group of a src is determined by its core (contiguous core pairs)
    GROUP_CORES_PRE = NCORES // n_groups
    src_grp = (src // NPC) // GROUP_CORES_PRE
    gdeg = np.zeros((N, n_groups), dtype=np.int32)
    for g in range(n_groups):
        gdeg[:, g] = np.bincount(dst[src_grp == g], minlength=N)
    maxgdeg = gdeg.max(axis=1)

    slot_of_node = np.empty(N, dtype=np.int64)
    nodes_sorted = np.empty((NCORES, NPC), dtype=np.int64)
    for c in range(NCORES):
        nodes = np.arange(c * NPC, (c + 1) * NPC)
        order = np.argsort(-maxgdeg[nodes], kind="stable")
        nodes_sorted[c] = nodes[order]
        slot_of_node[nodes[order]] = np.arange(NPC)

    core_of_node = np.arange(N) // NPC
    ghpr = core_of_node * SLICE + slot_of_node          # global hp row

    ce = dst // NPC                                     # dst core
    ds = slot_of_node[dst]                              # dst slot
    sg = ghpr[src] // GRP_ROWS                          # src group
    sl = (ghpr[src] % GRP_ROWS).astype(np.int64)        # group-local src row
    ZROW = SLOTS                                        # group-local zero row

    # occurrence index k of each edge within its (core, group, dst-slot) bucket
    order = np.lexsort((ds, sg, ce))
    ce_s, ds_s, sg_s, sl_s = ce[order], ds[order], sg[order], sl[order]
    key = (ce_s * n_groups + sg_s) * SLOTS + ds_s
    newgrp = np.ones(E, dtype=bool)
    newgrp[1:] = key[1:] != key[:-1]
    first_idx = np.maximum.accumulate(np.where(newgrp, np.arange(E), 0))
    k_s = np.arange(E) - first_idx
    KMAX = int(k_s.max()) + 1

    # dense idx tables A[core][g, k, slot] = group-local src row (ZROW = hole)
    A = np.full((NCORES, n_groups, KMAX, SLOTS), ZROW, dtype=np.int16)
    A[ce_s, sg_s, k_s, ds_s] = sl_s.astype(np.int16)
    # per (c, g, k): prefix length = last slot with an entry + 1
    has = (A != ZROW)
    rev_any = has[:, :, :, ::-1]
    firstpos = np.argmax(rev_any, axis=3)
    anyrow = rev_any.any(axis=3)
    n_cgk = np.where(anyrow, SLOTS - firstpos, 0)       # [NCORES, G, KMAX]
    n_gk = n_cgk.max(axis=0)                            # [G, KMAX] (SPMD-shared)

    passes = []          # (g, k, col_offset, mblocks)
    blob_cols = 0
    for g in range(n_groups):
        for k in range(KMAX):
            n = int(n_gk[g, k])
            if n == 0:
                continue
            mb = (n + P - 1) // P
            passes.append((g, k, blob_cols, mb))
            blob_cols += 8 * mb                          # n_pad/16 columns

    blob = np.full((NCORES, P, blob_cols), ZROW, dtype=np.int16)
    for (g, k, off, mb) in passes:
        npad = mb * P
        C = npad // 16
        for c in range(NCORES):
            w = A[c, g, k, :npad].reshape(C, 16).T
            for grp in range(8):
                blob[c, 16 * grp : 16 * (grp + 1), off : off + C] = w

    # per-core dinv layout [128, MBLK] (slot s = m*128+p -> [p, m])
    dinv_core = np.ones((NCORES, SLOTS), dtype=np.float32)
    for c in range(NCORES):
        dinv_core[c, :NPC] = dinv[nodes_sorted[c]]
    dinv_pm = dinv_core.reshape(NCORES, MBLK, P).transpose(0, 2, 1).copy()

    # per-core transposed features [IN_DIM, SLOTS]
    xt = np.zeros((NCORES, IN_DIM, SLOTS), dtype=np.float32)
    for c in range(NCORES):
        xt[c, :, :NPC] = np.asarray(x[nodes_sorted[c]], dtype=np.float32).T

    m.N, m.E, m.IN_DIM = N, E, IN_DIM
    m.NPC, m.SLOTS, m.MBLK, m.SLICE = NPC, SLOTS, MBLK, SLICE
    m.n_groups, m.GRP_ROWS, m.KMAX = n_groups, GRP_ROWS, KMAX
    m.passes = passes
    m.blob_cols = blob_cols
    m.blob = blob
    m.dinv_pm = dinv_pm
    m.xt = xt
    m.nodes_sorted = nodes_sorted
    return m


def _build(m, HID, EMB_PAD, skip_gathers=False, skip_ag=False):
    """Build the Bass program (SPMD, identical across cores)."""
    nc = bacc.Bacc("TRN2", target_bir_lowering=False)
    IN_DIM, SLOTS, MBLK, SLICE = m.IN_DIM, m.SLOTS, m.MBLK, m.SLICE
    G, GRP_ROWS = m.n_groups, m.GRP_ROWS
    NFULL = NCORES * SLICE
    KC = IN_DIM // P  # input-feature chunks (2)

    xt_in = nc.dram_tensor("xt", [IN_DIM, SLOTS], F32, kind="ExternalInput")
    dinv_in = nc.dram_tensor("dinv", [P, MBLK], F32, kind="ExternalInput")
    idx_in = nc.dram_tensor("idx", [P, m.blob_cols], I16, kind="ExternalInput")
    w1_in = nc.dram_tensor("w1", [IN_DIM, HID], F32, kind="ExternalInput")
    w2_in = nc.dram_tensor("w2", [HID, HID], F32, kind="ExternalInput")
    w3_in = nc.dram_tensor("w3", [HID, EMB_PAD], F32, kind="ExternalInput")
    b1_in = nc.dram_tensor("b1", [P, HID], F32, kind="ExternalInput")
    b2_in = nc.dram_tensor("b2", [P, HID], F32, kind="ExternalInput")
    b3_in = nc.dram_tensor("b3", [P, EMB_PAD], F32, kind="ExternalInput")
    out_dr = nc.dram_tensor("out", [P, MBLK * EMB_PAD], F32, kind="ExternalOutput")

    with tile.TileContext(nc) as tc:
        with (
            tc.tile_pool(name="const", bufs=1) as cp,
            tc.tile_pool(name="aggp", bufs=1) as ap,
            tc.tile_pool(name="dram", bufs=1, space="DRAM") as dp,
            tc.tile_pool(name="xtp", bufs=4) as xp,
            tc.tile_pool(name="rp", bufs=3) as rp,
            tc.tile_pool(name="rtp", bufs=3) as rtp,
            tc.tile_pool(name="idxp", bufs=4) as ip,
            tc.tile_pool(name="stp", bufs=3) as stp,
            tc.tile_pool(name="ps", bufs=3, space="PSUM") as pp,
            tc.tile_pool(name="psT", bufs=2, space="PSUM") as ppt,
        ):
            # constants
            w1_sb = cp.tile([P, KC * HID], F32)
            for c in range(KC):
                nc.sync.dma_start(
                    w1_sb[:, c * HID : (c + 1) * HID], w1_in[c * P : (c + 1) * P, :]
                )
            w2_sb = cp.tile([HID, HID], F32)
            nc.sync.dma_start(w2_sb[:], w2_in[:])
            w3_sb = cp.tile([HID, EMB_PAD], F32)
            nc.sync.dma_start(w3_sb[:], w3_in[:])
            b1_sb = cp.tile([P, HID], F32)
            nc.sync.dma_start(b1_sb[:], b1_in[:])
            b2_sb = cp.tile([P, HID], F32)
            nc.sync.dma_start(b2_sb[:], b2_in[:])
            b3_sb = cp.tile([P, EMB_PAD], F32)
            nc.sync.dma_start(b3_sb[:], b3_in[:])
            dinv_sb = cp.tile([P, MBLK], F32)
            nc.sync.dma_start(dinv_sb[:], dinv_in[:])
            ident = cp.tile([P, P], F32)
            make_identity(nc, ident[:])
            ztile = cp.tile([P, HID], F32)
            nc.vector.memset(ztile[:], 0.0)

            agg = ap.tile([P, MBLK * HID], F32)
            hp_sb = ap.tile([P, MBLK * HID], F32)
            out_sb = ap.tile([P, MBLK * EMB_PAD], F32)

            BF16 = mybir.dt.bfloat16
            hp_loc = [dp.tile([SLICE, HID], BF16, name=f"hploc{l}") for l in range(3)]
            hp_full_bf = [
                dp.tile([NFULL, HID], BF16, addr_space="Shared", name=f"hpfullb{l}")
                for l in range(3)
            ]
            hp_full = [
                dp.tile([NFULL, HID], F32, name=f"hpfull{l}") for l in range(3)
            ]
            zbf = cp.tile([P, HID], BF16)
            nc.vector.memset(zbf[:], 0.0)
            NEXP = NFULL // P

            def allgather(l):
                # fp32 -> bf16 cast during SWDGE store (halves the AG payload)
                nc.gpsimd.dma_start(
                    hp_loc[l][:SLOTS, :].rearrange("(m p) d -> p m d", p=P),
                    hp_sb[:].rearrange("p (m d) -> p m d", d=HID),
                )
                nc.sync.dma_start(hp_loc[l][SLOTS:SLICE, :], zbf[:])
                if not skip_ag:
                    nc.gpsimd.collective_compute(
                        "AllGather",
                        mybir.AluOpType.bypass,
                        replica_groups=[list(range(NCORES))],
                        ins=[hp_loc[l].opt()],
                        outs=[hp_full_bf[l].opt()],
                    )
                else:
                    nc.gpsimd.dma_start(hp_full_bf[l][:SLICE, :], hp_loc[l][:, :])
                # chunked local upcast bf16 -> fp32 for the 256B-row gathers
                # (a single 26MB cast DMA crashes the device; ~1.6MB chunks ok)
                nch = 16
                step = NEXP // nch
                for cc in range(nch):
                    rsl = slice(cc * step, NEXP if cc == nch - 1 else (cc + 1) * step)
                    nc.gpsimd.dma_start(
                        hp_full[l][:].rearrange("(p n) d -> p n d", p=P)[:, rsl, :],
                        hp_full_bf[l][:].rearrange("(p n) d -> p n d", p=P)[:, rsl, :],
                    )

            def aggregate(l):
                # agg starts as hp_sb (self-loop h'), add gathered neighbor rows
                nc.vector.tensor_copy(agg[:], hp_sb[:])
                if skip_gathers:
                    return
                for (g, k, off, mb) in m.passes:
                    n = mb * P
                    it = ip.tile([P, 8 * mb], I16, tag="idx")
                    nc.gpsimd.dma_start(it[:], idx_in[:, off : off + 8 * mb])
                    st = stp.tile([P, mb * HID], F32, tag="stg")
                    nc.gpsimd.dma_gather(
                        out_ap=st[:].rearrange("p (m d) -> p m d", d=HID),
                        in_ap=hp_full[l][g * GRP_ROWS : (g + 1) * GRP_ROWS, :],
                        idxs_ap=it[:],
                        num_idxs=n,
                        num_idxs_reg=n,
                        elem_size=HID,
                        single_packet=False,
                    )
                    nc.vector.tensor_add(
                        out=agg[:, : mb * HID], in0=agg[:, : mb * HID], in1=st[:]
                    )

            # ---- layer 1 local matmul: hp_sb = (x @ W1) * dinv
            for t in range(MBLK):
                ps = pp.tile([P, HID], F32, tag="ps")
                for c in range(KC):
                    xa = xp.tile([P, P], F32, tag="xt")
                    nc.sync.dma_start(
                        xa[:], xt_in[c * P : (c + 1) * P, t * P : (t + 1) * P]
                    )
                    nc.tensor.matmul(
                        ps[:],
                        lhsT=xa[:],
                        rhs=w1_sb[:, c * HID : (c + 1) * HID],
                        start=(c == 0),
                        stop=(c == KC - 1),
                    )
                nc.vector.tensor_scalar_mul(
                    hp_sb[:, t * HID : (t + 1) * HID], ps[:], dinv_sb[:, t : t + 1]
                )
            allgather(0)
            aggregate(0)

            # ---- layer 2: r = relu(dinv*agg + b1); hp_sb = (r @ W2) * dinv
            for t in range(MBLK):
                sl = slice(t * HID, (t + 1) * HID)
                r = rp.tile([P, HID], F32, tag="r")
                nc.vector.tensor_scalar_mul(r[:], agg[:, sl], dinv_sb[:, t : t + 1])
                nc.vector.tensor_add(out=r[:], in0=r[:], in1=b1_sb[:])
                nc.scalar.activation(r[:], r[:], mybir.ActivationFunctionType.Relu)
                pst = ppt.tile([HID, P], F32, tag="pst")
                nc.tensor.transpose(pst[:], r[:], ident[:])
                rT = rtp.tile([HID, P], F32, tag="rT")
                nc.vector.tensor_copy(rT[:], pst[:])
                ps = pp.tile([P, HID], F32, tag="ps")
                nc.tensor.matmul(ps[:], lhsT=rT[:], rhs=w2_sb[:], start=True, stop=True)
                nc.vector.tensor_scalar_mul(
                    hp_sb[:, sl], ps[:], dinv_sb[:, t : t + 1]
                )
            allgather(1)
            aggregate(1)

            # ---- layer 3 pre: hp_sb = relu(dinv*agg + b2) * dinv
            for t in range(MBLK):
                sl = slice(t * HID, (t + 1) * HID)
                r = rp.tile([P, HID], F32, tag="r")
                nc.vector.tensor_scalar_mul(r[:], agg[:, sl], dinv_sb[:, t : t + 1])
                nc.vector.tensor_add(out=r[:], in0=r[:], in1=b2_sb[:])
                nc.scalar.activation(r[:], r[:], mybir.ActivationFunctionType.Relu)
                nc.vector.tensor_scalar_mul(hp_sb[:, sl], r[:], dinv_sb[:, t : t + 1])
            allgather(2)
            aggregate(2)

            # ---- layer 3 final: out = (dinv*agg) @ W3 + b3
            for t in range(MBLK):
                sl = slice(t * HID, (t + 1) * HID)
                gm = rp.tile([P, HID], F32, tag="r")
                nc.vector.tensor_scalar_mul(gm[:], agg[:, sl], dinv_sb[:, t : t + 1])
                pst = ppt.tile([HID, P], F32, tag="pst")
                nc.tensor.transpose(pst[:], gm[:], ident[:])
                gT = rtp.tile([HID, P], F32, tag="rT")
                nc.vector.tensor_copy(gT[:], pst[:])
                ps3 = pp.tile([P, EMB_PAD], F32, tag="ps")
                nc.tensor.matmul(ps3[:], lhsT=gT[:], rhs=w3_sb[:], start=True, stop=True)
                nc.vector.tensor_add(
                    out=out_sb[:, t * EMB_PAD : (t + 1) * EMB_PAD],
                    in0=ps3[:],
                    in1=b3_sb[:],
                )
            nc.sync.dma_start(out_dr[:], out_sb[:])
    nc.finalize()
    return nc


_CACHE = {}


def kernel(x, edge_index, W1, b1, W2, b2, W3, b3):
    x = np.asarray(x)
    edge_index = np.asarray(edge_index)
    N, IN_DIM = x.shape
    HID = W1.shape[1]
    EMB = W3.shape[1]
    EMB_PAD = 16

    m = _preprocess(x, edge_index)

    key = (N, IN_DIM, HID, EMB, tuple(mb for (_, _, _, mb) in m.passes))
    if key not in _CACHE:
        _CACHE[key] = _build(m, HID, EMB_PAD)
    nc = _CACHE[key]

    w3p = np.zeros((HID, EMB_PAD), dtype=np.float32)
    w3p[:, :EMB] = np.asarray(W3, dtype=np.float32)
    b3p = np.zeros((EMB_PAD,), dtype=np.float32)
    b3p[:EMB] = np.asarray(b3, dtype=np.float32)

    def rep(v, d):
        return np.broadcast_to(np.asarray(v, np.float32).reshape(1, d), (P, d)).copy()

    in_maps = []
    for c in range(NCORES):
        in_maps.append(
            {
                "xt": m.xt[c],
                "dinv": m.dinv_pm[c],
                "idx": m.blob[c],
                "w1": np.asarray(W1, np.float32),
                "w2": np.asarray(W2, np.float32),
                "w3": w3p,
                "b1": rep(b1, HID),
                "b2": rep(b2, HID),
                "b3": rep(b3p, EMB_PAD),
            }
        )

    from concourse.bass_utils import run_bass_kernel_spmd

    res = run_bass_kernel_spmd(nc, in_maps, core_ids=list(range(NCORES)))

    out = np.empty((N, EMB), dtype=np.float32)
    for c in range(NCORES):
        o = res.results[c]["out"].reshape(P, m.MBLK, EMB_PAD)
        o = o.transpose(1, 0, 2).reshape(m.SLOTS, EMB_PAD)[: m.NPC, :EMB]
        out[m.nodes_sorted[c]] = o
    return out


def _make_in_maps(m, W1, b1, W2, b2, W3, b3, HID, EMB, EMB_PAD):
    w3p = np.zeros((HID, EMB_PAD), dtype=np.float32)
    w3p[:, :EMB] = np.asarray(W3, dtype=np.float32)
    b3p = np.zeros((EMB_PAD,), dtype=np.float32)
    b3p[:EMB] = np.asarray(b3, dtype=np.float32)

    def rep(v, d):
        return np.broadcast_to(np.asarray(v, np.float32).reshape(1, d), (P, d)).copy()

    return [
        {
            "xt": m.xt[c],
            "dinv": m.dinv_pm[c],
            "idx": m.blob[c],
            "w1": np.asarray(W1, np.float32),
            "w2": np.asarray(W2, np.float32),
            "w3": w3p,
            "b1": rep(b1, HID),
            "b2": rep(b2, HID),
            "b3": rep(b3p, EMB_PAD),
        }
        for c in range(NCORES)
    ]


def _timed_sharded_run(nc, in_maps, iters):
    """Upload once, run the jitted sharded executable repeatedly; min wall (s)."""
    import time
    import jax
    from jax.sharding import Mesh, PartitionSpec
    from jax.experimental.shard_map import shard_map
    from concourse.bass2jax import (
        _bass_exec_p,
        install_neuronx_cc_hook,
        partition_id_tensor,
    )

    install_neuronx_cc_hook()
    partition_name = nc.partition_id_tensor.name if nc.partition_id_tensor else None
    in_names, out_names, out_avals, zero_outs = [], [], [], []
    for alloc in nc.m.functions[0].allocations:
        if not isinstance(alloc, mybir.MemoryLocationSet):
            continue
        name = alloc.memorylocations[0].name
        if alloc.kind == "ExternalInput":
            if name != partition_name:
                in_names.append(name)
        elif alloc.kind == "ExternalOutput":
            out_names.append(name)
            shape = tuple(alloc.tensor_shape)
            dtype = mybir.dt.np(alloc.dtype)
            out_avals.append(jax.core.ShapedArray(shape, dtype))
            zero_outs.append(np.zeros(shape, dtype))
    n_params = len(in_names)
    all_names = in_names + out_names + ([partition_name] if partition_name else [])

    def _body(*args):
        operands = list(args)
        if partition_name is not None:
            operands.append(partition_id_tensor())
        outs = _bass_exec_p.bind(
            *operands,
            out_avals=tuple(out_avals),
            in_names=tuple(all_names),
            out_names=tuple(out_names),
            lowering_input_output_aliases=(),
            sim_require_finite=True,
            sim_require_nnan=True,
            nc=nc,
        )
        return tuple(outs)

    devices = jax.devices()[:NCORES]
    mesh = Mesh(np.asarray(devices), ("core",))
    n_outs = len(out_avals)
    fn = jax.jit(
        shard_map(
            _body,
            mesh=mesh,
            in_specs=(PartitionSpec("core"),) * (n_params + n_outs),
            out_specs=(PartitionSpec("core"),) * n_outs,
            check_rep=False,
        ),
        keep_unused=True,
    )
    concat = [
        np.concatenate([in_maps[c][n] for c in range(NCORES)], axis=0)
        for n in in_names
    ]
    zeros = [np.zeros((NCORES * z.shape[0], *z.shape[1:]), z.dtype) for z in zero_outs]
    args = [jax.device_put(a) for a in concat + zeros]
    o = fn(*args)
    jax.block_until_ready(o)
    times = []
    for _ in range(iters):
        t0 = time.perf_counter()
        o = fn(*args)
        jax.block_until_ready(o)
        times.append(time.perf_counter() - t0)
    return min(times)


def _build_noop():
    nc = bacc.Bacc("TRN2", target_bir_lowering=False)
    a = nc.dram_tensor("a", [P, 64], F32, kind="ExternalInput")
    o = nc.dram_tensor("o", [P, 64], F32, kind="ExternalOutput")
    with tile.TileContext(nc) as tc:
        with tc.tile_pool(name="sb", bufs=1) as sb:
            t = sb.tile([P, 64], F32)
            nc.sync.dma_start(t[:], a[:])
            nc.sync.dma_start(o[:], t[:])
    nc.finalize()
    return nc


def estimate_hw_time(inputs, iters=6):
    """Device-resident repeated execution minus the axon dispatch floor."""
    x = np.asarray(inputs["x"])
    edge_index = np.asarray(inputs["edge_index"])
    HID = inputs["W1"].shape[1]
    EMB = inputs["W3"].shape[1]
    EMB_PAD = 16
    m = _preprocess(x, edge_index)
    key = (x.shape[0], x.shape[1], HID, EMB, tuple(mb for (_, _, _, mb) in m.passes))
    if key not in _CACHE:
        _CACHE[key] = _build(m, HID, EMB_PAD)
    nc = _CACHE[key]
    in_maps = _make_in_maps(
        m, inputs["W1"], inputs["b1"], inputs["W2"], inputs["b2"],
        inputs["W3"], inputs["b3"], HID, EMB, EMB_PAD,
    )
    t_full = _timed_sharded_run(nc, in_maps, iters)
    noop = _build_noop()
    noop_maps = [{"a": np.zeros((P, 64), np.float32)} for _ in range(NCORES)]
    t_floor = _timed_sharded_run(noop, noop_maps, iters)
    print(f"(wall/exec: {t_full*1e3:.1f} ms, dispatch floor: {t_floor*1e3:.1f} ms)")
    return max(t_full - t_floor, 0.0) * 1e9


# revision 14
# speedup vs baseline: 1.0968x; 1.0968x over previous
"""3-layer GCN encoder on 8 TRN2 NeuronCores (Bass/Tile).

Strategy: partition nodes (dst) across 8 cores. Per layer: local matmul
h = prev @ W, pre-scale h' = h * dinv (factors the symmetric norm so the
edge aggregation is an unweighted sum), AllGather h' (fp32 [101376, 64]),
then aggregate in-edges with bulk dma_gather passes: pass (g, k) gathers
the k-th neighbor-in-source-group-g of every node slot (nodes degree-
sorted so each pass covers a slot prefix; holes point at a zero row).
DVE accumulates gathered rows into the fp32 agg tile. Finally
out[d] = dinv[d] * (h'[d] + sum) -> bias/relu -> next layer.
"""
import os
import numpy as np

import concourse.bacc as bacc
import concourse.bass as bass
import concourse.mybir as mybir
import concourse.tile as tile
from concourse.masks import make_identity

F32 = mybir.dt.float32
I16 = mybir.dt.int16

NCORES = 8
P = 128


class Meta:
    pass


def _preprocess(x, edge_index, n_groups=4):
    """Host-side graph preprocessing -> per-core arrays + pass structure."""
    m = Meta()
    N, IN_DIM = x.shape
    E = edge_index.shape[1]
    NPC = N // NCORES                       # real nodes per core
    SLOTS = ((NPC + P - 1) // P) * P        # padded slots per core (mult of 128)
    MBLK = SLOTS // P                       # 98 column blocks
    SLICE = SLOTS + P                       # hp rows per core (+128 zero rows)
    GROUP_CORES = NCORES // n_groups
    GRP_ROWS = GROUP_CORES * SLICE          # hp rows per source group
    assert GRP_ROWS - 1 < 32768, "group rows must fit int16"

    src = np.asarray(edge_index[0], dtype=np.int64)
    dst = np.asarray(edge_index[1], dtype=np.int64)
    deg = np.bincount(dst, minlength=N)
    dinv = 1.0 / np.sqrt(deg + 1.0)

    # per-core slot assignment sorted by MAX per-source-group in-degree:
    # every group-g pass prefix is then bounded by the maxdeg-k prefix,
    # which minimizes zero-fill holes across all groups at once.
    core_of_node_pre = np.arange(N) // NPC
    # group of a src is determined by its core (contiguous core pairs)
    GROUP_CORES_PRE = NCORES // n_groups
    src_grp = (src // NPC) // GROUP_CORES_PRE
    gdeg = np.zeros((N, n_groups), dtype=np.int32)
    for g in range(n_groups):
        gdeg[:, g] = np.bincount(dst[src_grp == g], minlength=N)
    maxgdeg = gdeg.max(axis=1)

    slot_of_node = np.empty(N, dtype=np.int64)
    nodes_sorted = np.empty((NCORES, NPC), dtype=np.int64)
    for c in range(NCORES):
        nodes = np.arange(c * NPC, (c + 1) * NPC)
        order = np.argsort(-maxgdeg[nodes], kind="stable")
        nodes_sorted[c] = nodes[order]
        slot_of_node[nodes[order]] = np.arange(NPC)

    core_of_node = np.arange(N) // NPC
    ghpr = core_of_node * SLICE + slot_of_node          # global hp row

    ce = dst // NPC                                     # dst core
    ds = slot_of_node[dst]                              # dst slot
    sg = ghpr[src] // GRP_ROWS                          # src group
    sl = (ghpr[src] % GRP_ROWS).astype(np.int64)        # group-local src row
    ZROW = SLOTS                                        # group-local zero row

    # occurrence index k of each edge within its (core, group, dst-slot) bucket
    order = np.lexsort((ds, sg, ce))
    ce_s, ds_s, sg_s, sl_s = ce[order], ds[order], sg[order], sl[order]
    key = (ce_s * n_groups + sg_s) * SLOTS + ds_s
    newgrp = np.ones(E, dtype=bool)
    newgrp[1:] = key[1:] != key[:-1]
    first_idx = np.maximum.accumulate(np.where(newgrp, np.arange(E), 0))
    k_s = np.arange(E) - first_idx
    KMAX = int(k_s.max()) + 1

    # dense idx tables A[core][g, k, slot] = group-local src row (ZROW = hole)
    A = np.full((NCORES, n_groups, KMAX, SLOTS), ZROW, dtype=np.int16)
    A[ce_s, sg_s, k_s, ds_s] = sl_s.astype(np.int16)
    # per (c, g, k): prefix length = last slot with an entry + 1
    has = (A != ZROW)
    rev_any = has[:, :, :, ::-1]
    firstpos = np.argmax(rev_any, axis=3)
    anyrow = rev_any.any(axis=3)
    n_cgk = np.where(anyrow, SLOTS - firstpos, 0)       # [NCORES, G, KMAX]
    n_gk = n_cgk.max(axis=0)                            # [G, KMAX] (SPMD-shared)

    passes = []          # (g, k, col_offset, mblocks)
    blob_cols = 0
    for g in range(n_groups):
        for k in range(KMAX):
            n = int(n_gk[g, k])
            if n == 0:
                continue
            mb = (n + P - 1) // P
            passes.append((g, k, blob_cols, mb))
            blob_cols += 8 * mb                          # n_pad/16 columns

    blob = np.full((NCORES, P, blob_cols), ZROW, dtype=np.int16)
    for (g, k, off, mb) in passes:
        npad = mb * P
        C = npad // 16
        for c in range(NCORES):
            w = A[c, g, k, :npad].reshape(C, 16).T
            for grp in range(8):
                blob[c, 16 * grp : 16 * (grp + 1), off : off + C] = w

    # per-core dinv layout [128, MBLK] (slot s = m*128+p -> [p, m])
    dinv_core = np.ones((NCORES, SLOTS), dtype=np.float32)
    for c in range(NCORES):
        dinv_core[c, :NPC] = dinv[nodes_sorted[c]]
    dinv_pm = dinv_core.reshape(NCORES, MBLK, P).transpose(0, 2, 1).copy()

    # per-core transposed features [IN_DIM, SLOTS]
    xt = np.zeros((NCORES, IN_DIM, SLOTS), dtype=np.float32)
    for c in range(NCORES):
        xt[c, :, :NPC] = np.asarray(x[nodes_sorted[c]], dtype=np.float32).T

    m.N, m.E, m.IN_DIM = N, E, IN_DIM
    m.NPC, m.SLOTS, m.MBLK, m.SLICE = NPC, SLOTS, MBLK, SLICE
    m.n_groups, m.GRP_ROWS, m.KMAX = n_groups, GRP_ROWS, KMAX
    m.passes = passes
    m.blob_cols = blob_cols
    m.blob = blob
    m.dinv_pm = dinv_pm
    m.xt = xt
    m.nodes_sorted = nodes_sorted
    return m


def _build(m, HID, EMB_PAD, skip_gathers=False, skip_ag=False):
    """Build the Bass program (SPMD, identical across cores)."""
    nc = bacc.Bacc("TRN2", target_bir_lowering=False)
    IN_DIM, SLOTS, MBLK, SLICE = m.IN_DIM, m.SLOTS, m.MBLK, m.SLICE
    G, GRP_ROWS = m.n_groups, m.GRP_ROWS
    NFULL = NCORES * SLICE
    KC = IN_DIM // P  # input-feature chunks (2)

    xt_in = nc.dram_tensor("xt", [IN_DIM, SLOTS], F32, kind="ExternalInput")
    dinv_in = nc.dram_tensor("dinv", [P, MBLK], F32, kind="ExternalInput")
    idx_in = nc.dram_tensor("idx", [P, m.blob_cols], I16, kind="ExternalInput")
    w1_in = nc.dram_tensor("w1", [IN_DIM, HID], F32, kind="ExternalInput")
    w2_in = nc.dram_tensor("w2", [HID, HID], F32, kind="ExternalInput")
    w3_in = nc.dram_tensor("w3", [HID, EMB_PAD], F32, kind="ExternalInput")
    b1_in = nc.dram_tensor("b1", [P, HID], F32, kind="ExternalInput")
    b2_in = nc.dram_tensor("b2", [P, HID], F32, kind="ExternalInput")
    b3_in = nc.dram_tensor("b3", [P, EMB_PAD], F32, kind="ExternalInput")
    out_dr = nc.dram_tensor("out", [P, MBLK * EMB_PAD], F32, kind="ExternalOutput")

    with tile.TileContext(nc) as tc:
        with (
            tc.tile_pool(name="const", bufs=1) as cp,
            tc.tile_pool(name="aggp", bufs=1) as ap,
            tc.tile_pool(name="dram", bufs=1, space="DRAM") as dp,
            tc.tile_pool(name="xtp", bufs=4) as xp,
            tc.tile_pool(name="rp", bufs=3) as rp,
            tc.tile_pool(name="rtp", bufs=3) as rtp,
            tc.tile_pool(name="idxp", bufs=2) as ip,
            tc.tile_pool(name="stp", bufs=2) as stp,
            tc.tile_pool(name="ps", bufs=3, space="PSUM") as pp,
            tc.tile_pool(name="psT", bufs=2, space="PSUM") as ppt,
        ):
            # constants
            w1_sb = cp.tile([P, KC * HID], F32)
            for c in range(KC):
                nc.sync.dma_start(
                    w1_sb[:, c * HID : (c + 1) * HID], w1_in[c * P : (c + 1) * P, :]
                )
            w2_sb = cp.tile([HID, HID], F32)
            nc.sync.dma_start(w2_sb[:], w2_in[:])
            w3_sb = cp.tile([HID, EMB_PAD], F32)
            nc.sync.dma_start(w3_sb[:], w3_in[:])
            b1_sb = cp.tile([P, HID], F32)
            nc.sync.dma_start(b1_sb[:], b1_in[:])
            b2_sb = cp.tile([P, HID], F32)
            nc.sync.dma_start(b2_sb[:], b2_in[:])
            b3_sb = cp.tile([P, EMB_PAD], F32)
            nc.sync.dma_start(b3_sb[:], b3_in[:])
            dinv_sb = cp.tile([P, MBLK], F32)
            nc.sync.dma_start(dinv_sb[:], dinv_in[:])
            ident = cp.tile([P, P], F32)
            make_identity(nc, ident[:])
            ztile = cp.tile([P, HID], F32)
            nc.vector.memset(ztile[:], 0.0)

            agg = ap.tile([P, MBLK * HID], F32)
            hp_sb = ap.tile([P, MBLK * HID], F32)
            out_sb = ap.tile([P, MBLK * EMB_PAD], F32)

            BF16 = mybir.dt.bfloat16
            hp_loc = [dp.tile([SLICE, HID], BF16, name=f"hploc{l}") for l in range(3)]
            hp_full_bf = [
                dp.tile([NFULL, HID], BF16, addr_space="Shared", name=f"hpfullb{l}")
                for l in range(3)
            ]
            hp_full = [
                dp.tile([NFULL, HID], F32, name=f"hpfull{l}") for l in range(3)
            ]
            zbf = cp.tile([P, HID], BF16)
            nc.vector.memset(zbf[:], 0.0)
            NEXP = NFULL // P

            def allgather(l):
                # fp32 -> bf16 cast during SWDGE store (halves the AG payload)
                nc.gpsimd.dma_start(
                    hp_loc[l][:SLOTS, :].rearrange("(m p) d -> p m d", p=P),
                    hp_sb[:].rearrange("p (m d) -> p m d", d=HID),
                )
                nc.sync.dma_start(hp_loc[l][SLOTS:SLICE, :], zbf[:])
                if not skip_ag:
                    nc.gpsimd.collective_compute(
                        "AllGather",
                        mybir.AluOpType.bypass,
                        replica_groups=[list(range(NCORES))],
                        ins=[hp_loc[l].opt()],
                        outs=[hp_full_bf[l].opt()],
                    )
                else:
                    nc.gpsimd.dma_start(hp_full_bf[l][:SLICE, :], hp_loc[l][:, :])
                # chunked local upcast bf16 -> fp32 for the 256B-row gathers
                # (a single 26MB cast DMA crashes the device; ~1.6MB chunks ok)
                nch = 16
                step = NEXP // nch
                for cc in range(nch):
                    rsl = slice(cc * step, NEXP if cc == nch - 1 else (cc + 1) * step)
                    nc.gpsimd.dma_start(
                        hp_full[l][:].rearrange("(p n) d -> p n d", p=P)[:, rsl, :],
                        hp_full_bf[l][:].rearrange("(p n) d -> p n d", p=P)[:, rsl, :],
                    )

            def aggregate(l):
                # agg starts as hp_sb (self-loop h'), add gathered neighbor rows
                nc.vector.tensor_copy(agg[:], hp_sb[:])
                if skip_gathers:
                    return
                for (g, k, off, mb) in m.passes:
                    n = mb * P
                    it = ip.tile([P, 8 * mb], I16, tag="idx")
                    nc.gpsimd.dma_start(it[:], idx_in[:, off : off + 8 * mb])
                    st = stp.tile([P, mb * HID], F32, tag="stg")
                    nc.gpsimd.dma_gather(
                        out_ap=st[:].rearrange("p (m d) -> p m d", d=HID),
                        in_ap=hp_full[l][g * GRP_ROWS : (g + 1) * GRP_ROWS, :],
                        idxs_ap=it[:],
                        num_idxs=n,
                        num_idxs_reg=n,
                        elem_size=HID,
                        single_packet=False,
                    )
                    nc.vector.tensor_add(
                        out=agg[:, : mb * HID], in0=agg[:, : mb * HID], in1=st[:]
                    )

            # ---- layer 1 local matmul: hp_sb = (x @ W1) * dinv
            for t in range(MBLK):
                ps = pp.tile([P, HID], F32, tag="ps")
                for c in range(KC):
                    xa = xp.tile([P, P], F32, tag="xt")
                    nc.sync.dma_start(
                        xa[:], xt_in[c * P : (c + 1) * P, t * P : (t + 1) * P]
                    )
                    nc.tensor.matmul(
                        ps[:],
                        lhsT=xa[:],
                        rhs=w1_sb[:, c * HID : (c + 1) * HID],
                        start=(c == 0),
                        stop=(c == KC - 1),
                    )
                nc.vector.tensor_scalar_mul(
                    hp_sb[:, t * HID : (t + 1) * HID], ps[:], dinv_sb[:, t : t + 1]
                )
            allgather(0)
            aggregate(0)

            # ---- layer 2: r = relu(dinv*agg + b1); hp_sb = (r @ W2) * dinv
            for t in range(MBLK):
                sl = slice(t * HID, (t + 1) * HID)
                r = rp.tile([P, HID], F32, tag="r")
                nc.vector.tensor_scalar_mul(r[:], agg[:, sl], dinv_sb[:, t : t + 1])
                nc.vector.tensor_add(out=r[:], in0=r[:], in1=b1_sb[:])
                nc.scalar.activation(r[:], r[:], mybir.ActivationFunctionType.Relu)
                pst = ppt.tile([HID, P], F32, tag="pst")
                nc.tensor.transpose(pst[:], r[:], ident[:])
                rT = rtp.tile([HID, P], F32, tag="rT")
                nc.vector.tensor_copy(rT[:], pst[:])
                ps = pp.tile([P, HID], F32, tag="ps")
                nc.tensor.matmul(ps[:], lhsT=rT[:], rhs=w2_sb[:], start=True, stop=True)
                nc.vector.tensor_scalar_mul(
                    hp_sb[:, sl], ps[:], dinv_sb[:, t : t + 1]
                )
            allgather(1)
            aggregate(1)

            # ---- layer 3 pre: hp_sb = relu(dinv*agg + b2) * dinv
            for t in range(MBLK):
                sl = slice(t * HID, (t + 1) * HID)
                r = rp.tile([P, HID], F32, tag="r")
                nc.vector.tensor_scalar_mul(r[:], agg[:, sl], dinv_sb[:, t : t + 1])
                nc.vector.tensor_add(out=r[:], in0=r[:], in1=b2_sb[:])
                nc.scalar.activation(r[:], r[:], mybir.ActivationFunctionType.Relu)
                nc.vector.tensor_scalar_mul(hp_sb[:, sl], r[:], dinv_sb[:, t : t + 1])
            allgather(2)
            aggregate(2)

            # ---- layer 3 final: out = (dinv*agg) @ W3 + b3
            for t in range(MBLK):
                sl = slice(t * HID, (t + 1) * HID)
                gm = rp.tile([P, HID], F32, tag="r")
                nc.vector.tensor_scalar_mul(gm[:], agg[:, sl], dinv_sb[:, t : t + 1])
                pst = ppt.tile([HID, P], F32, tag="pst")
                nc.tensor.transpose(pst[:], gm[:], ident[:])
                gT = rtp.tile([HID, P], F32, tag="rT")
                nc.vector.tensor_copy(gT[:], pst[:])
                ps3 = pp.tile([P, EMB_PAD], F32, tag="ps")
                nc.tensor.matmul(ps3[:], lhsT=gT[:], rhs=w3_sb[:], start=True, stop=True)
                nc.vector.tensor_add(
                    out=out_sb[:, t * EMB_PAD : (t + 1) * EMB_PAD],
                    in0=ps3[:],
                    in1=b3_sb[:],
                )
            nc.sync.dma_start(out_dr[:], out_sb[:])
    nc.finalize()
    return nc


_CACHE = {}


def kernel(x, edge_index, W1, b1, W2, b2, W3, b3):
    x = np.asarray(x)
    edge_index = np.asarray(edge_index)
    N, IN_DIM = x.shape
    HID = W1.shape[1]
    EMB = W3.shape[1]
    EMB_PAD = 16

    m = _preprocess(x, edge_index)

    key = (N, IN_DIM, HID, EMB, tuple(mb for (_, _, _, mb) in m.passes))
    if key not in _CACHE:
        _CACHE[key] = _build(m, HID, EMB_PAD)
    nc = _CACHE[key]

    w3p = np.zeros((HID, EMB_PAD), dtype=np.float32)
    w3p[:, :EMB] = np.asarray(W3, dtype=np.float32)
    b3p = np.zeros((EMB_PAD,), dtype=np.float32)
    b3p[:EMB] = np.asarray(b3, dtype=np.float32)

    def rep(v, d):
        return np.broadcast_to(np.asarray(v, np.float32).reshape(1, d), (P, d)).copy()

    in_maps = []
    for c in range(NCORES):
        in_maps.append(
            {
                "xt": m.xt[c],
                "dinv": m.dinv_pm[c],
                "idx": m.blob[c],
                "w1": np.asarray(W1, np.float32),
                "w2": np.asarray(W2, np.float32),
                "w3": w3p,
                "b1": rep(b1, HID),
                "b2": rep(b2, HID),
                "b3": rep(b3p, EMB_PAD),
            }
        )

    from concourse.bass_utils import run_bass_kernel_spmd

    res = run_bass_kernel_spmd(nc, in_maps, core_ids=list(range(NCORES)))

    out = np.empty((N, EMB), dtype=np.float32)
    for c in range(NCORES):
        o = res.results[c]["out"].reshape(P, m.MBLK, EMB_PAD)
        o = o.transpose(1, 0, 2).reshape(m.SLOTS, EMB_PAD)[: m.NPC, :EMB]
        out[m.nodes_sorted[c]] = o
    return out


def _make_in_maps(m, W1, b1, W2, b2, W3, b3, HID, EMB, EMB_PAD):
    w3p = np.zeros((HID, EMB_PAD), dtype=np.float32)
    w3p[:, :EMB] = np.asarray(W3, dtype=np.float32)
    b3p = np.zeros((EMB_PAD,), dtype=np.float32)
    b3p[:EMB] = np.asarray(b3, dtype=np.float32)

    def rep(v, d):
        return np.broadcast_to(np.asarray(v, np.float32).reshape(1, d), (P, d)).copy()

    return [
        {
            "xt": m.xt[c],
            "dinv": m.dinv_pm[c],
            "idx": m.blob[c],
            "w1": np.asarray(W1, np.float32),
            "w2": np.asarray(W2, np.float32),
            "w3": w3p,
            "b1": rep(b1, HID),
            "b2": rep(b2, HID),
            "b3": rep(b3p, EMB_PAD),
        }
        for c in range(NCORES)
    ]


def _timed_sharded_run(nc, in_maps, iters):
    """Upload once, run the jitted sharded executable repeatedly; min wall (s)."""
    import time
    import jax
    from jax.sharding import Mesh, PartitionSpec
    from jax.experimental.shard_map import shard_map
    from concourse.bass2jax import (
        _bass_exec_p,
        install_neuronx_cc_hook,
        partition_id_tensor,
    )

    install_neuronx_cc_hook()
    partition_name = nc.partition_id_tensor.name if nc.partition_id_tensor else None
    in_names, out_names, out_avals, zero_outs = [], [], [], []
    for alloc in nc.m.functions[0].allocations:
        if not isinstance(alloc, mybir.MemoryLocationSet):
            continue
        name = alloc.memorylocations[0].name
        if alloc.kind == "ExternalInput":
            if name != partition_name:
                in_names.append(name)
        elif alloc.kind == "ExternalOutput":
            out_names.append(name)
            shape = tuple(alloc.tensor_shape)
            dtype = mybir.dt.np(alloc.dtype)
            out_avals.append(jax.core.ShapedArray(shape, dtype))
            zero_outs.append(np.zeros(shape, dtype))
    n_params = len(in_names)
    all_names = in_names + out_names + ([partition_name] if partition_name else [])

    def _body(*args):
        operands = list(args)
        if partition_name is not None:
            operands.append(partition_id_tensor())
        outs = _bass_exec_p.bind(
            *operands,
            out_avals=tuple(out_avals),
            in_names=tuple(all_names),
            out_names=tuple(out_names),
            lowering_input_output_aliases=(),
            sim_require_finite=True,
            sim_require_nnan=True,
            nc=nc,
        )
        return tuple(outs)

    devices = jax.devices()[:NCORES]
    mesh = Mesh(np.asarray(devices), ("core",))
    n_outs = len(out_avals)
    fn = jax.jit(
        shard_map(
            _body,
            mesh=mesh,
            in_specs=(PartitionSpec("core"),) * (n_params + n_outs),
            out_specs=(PartitionSpec("core"),) * n_outs,
            check_rep=False,
        ),
        keep_unused=True,
    )
    concat = [
        np.concatenate([in_maps[c][n] for c in range(NCORES)], axis=0)
        for n in in_names
    ]
    zeros = [np.zeros((NCORES * z.shape[0], *z.shape[1:]), z.dtype) for z in zero_outs]
    args = [jax.device_put(a) for a in concat + zeros]
    o = fn(*args)
    jax.block_until_ready(o)
    times = []
    for _ in range(iters):
        t0 = time.perf_counter()
        o = fn(*args)
        jax.block_until_ready(o)
        times.append(time.perf_counter() - t0)
    return min(times)


def _build_noop():
    nc = bacc.Bacc("TRN2", target_bir_lowering=False)
    a = nc.dram_tensor("a", [P, 64], F32, kind="ExternalInput")
    o = nc.dram_tensor("o", [P, 64], F32, kind="ExternalOutput")
    with tile.TileContext(nc) as tc:
        with tc.tile_pool(name="sb", bufs=1) as sb:
            t = sb.tile([P, 64], F32)
            nc.sync.dma_start(t[:], a[:])
            nc.sync.dma_start(o[:], t[:])
    nc.finalize()
    return nc


def estimate_hw_time(inputs, iters=6):
    """Device-resident repeated execution minus the axon dispatch floor."""
    x = np.asarray(inputs["x"])
    edge_index = np.asarray(inputs["edge_index"])
    HID = inputs["W1"].shape[1]
    EMB = inputs["W3"].shape[1]
    EMB_PAD = 16
    m = _preprocess(x, edge_index)
    key = (x.shape[0], x.shape[1], HID, EMB, tuple(mb for (_, _, _, mb) in m.passes))
    if key not in _CACHE:
        _CACHE[key] = _build(m, HID, EMB_PAD)
    nc = _CACHE[key]
    in_maps = _make_in_maps(
        m, inputs["W1"], inputs["b1"], inputs["W2"], inputs["b2"],
        inputs["W3"], inputs["b3"], HID, EMB, EMB_PAD,
    )
    t_full = _timed_sharded_run(nc, in_maps, iters)
    noop = _build_noop()
    noop_maps = [{"a": np.zeros((P, 64), np.float32)} for _ in range(NCORES)]
    t_floor = _timed_sharded_run(noop, noop_maps, iters)
    print(f"(wall/exec: {t_full*1e3:.1f} ms, dispatch floor: {t_floor*1e3:.1f} ms)")
    return max(t_full - t_floor, 0.0) * 1e9
